# revision 1
# baseline (speedup 1.0000x reference)
"""Multi-head attention (B=2, S=2048, H=1024, NH=16, HD=64) on 8 trn2 cores.

Sharding: tensor-parallel over heads. Core c owns heads {2c, 2c+1}, i.e.
feature columns [128c, 128c+128) of q/k/v. Wq/Wk/Wv are column-sharded,
Wo row-sharded; each core computes a full-shape partial output and the
host sums the 8 partials (the row-parallel reduce) during unshard.

On-chip layout is feature-major ("transposed"): the host passes
hsT = hidden_states.T so both matmul operands of every projection have
the contraction dim on partitions and no on-chip transposes of big
tensors are needed. Attention works on scoresT[tk, tq]; softmax's
normalizer comes from a ones-column augmented V matmul (exp is safe
without max-subtraction because scores are O(6) here).

QKV and output projections run in float32r (fast fp32 mode, ~1.4e-4 rel
accuracy); score/ctx matmul operands are fp16; all accumulation is full
fp32 in PSUM. Attention matmuls are zero-padded to full 128x128 array
shapes (K=128 scores via zero-padded per-head K, M=128 ctx via padded
augmented-V) - half-array matmuls don't register as activity for the
PE's HAM clock gate and the whole phase runs at 1.2GHz otherwise.
"""

import numpy as np

B, S, H, NH, HD = 2, 2048, 1024, 16, 64
NCORES = 8
JC = 128  # head-columns per core (2 heads x 64)
T = B * S  # 4096 tokens
TQB = 512  # tq block
NKT = S // 128  # 16 tk blocks per batch
WAVE = 1024  # qkv projection token-chunk per wave
BASE = 10000.0

_nc_cache = [None]

_LDW_OPT = False


def _patch_ldw_opt():
    from concourse import bass_utils as _bu

    if getattr(_bu, "_ldw_patched", False):
        return
    _orig = _bu.run_command

    def _patched(argv, **kw):
        argv = [
            a.replace("--enable-ldw-opt=false", "--enable-ldw-opt=true")
            if _LDW_OPT and isinstance(a, str)
            else a
            for a in argv
        ]
        return _orig(argv, **kw)

    _bu.run_command = _patched
    _bu._ldw_patched = True


def _build():
    _patch_ldw_opt()
    import concourse.tile as tile
    from concourse import bacc, mybir
    from concourse.masks import make_identity

    F32 = mybir.dt.float32
    F32R = mybir.dt.float32r
    BF16 = mybir.dt.bfloat16
    F16 = mybir.dt.float16
    EXP = mybir.ActivationFunctionType.Exp

    nc = bacc.Bacc("TRN2", target_bir_lowering=False, debug=False)

    hsT = nc.dram_tensor("hsT", [H, T], F32R, kind="ExternalInput").ap()
    wqT = nc.dram_tensor("wqT", [H, JC], F32R, kind="ExternalInput").ap()
    wkT = nc.dram_tensor("wkT", [H, JC], F32R, kind="ExternalInput").ap()
    wvT = nc.dram_tensor("wvT", [H, JC], F32R, kind="ExternalInput").ap()
    woJI = nc.dram_tensor("woJI", [JC, H], F32R, kind="ExternalInput").ap()
    cosT = nc.dram_tensor("cosT", [128, S], F32, kind="ExternalInput").ap()
    sinTs = nc.dram_tensor("sinTs", [128, S], F32, kind="ExternalInput").ap()
    out = nc.dram_tensor("out", [T, H], F32, kind="ExternalOutput").ap()

    with tile.TileContext(nc) as tc:
        with (
            tc.tile_pool(name="wts", bufs=1) as wts,
            tc.tile_pool(name="tabs", bufs=1) as tabs,
            tc.tile_pool(name="hst", bufs=20) as hst,
            tc.tile_pool(name="qkv", bufs=2) as qkvp,
            tc.tile_pool(name="ps", bufs=3, space="PSUM") as ps,
            tc.tile_pool(name="cxp", bufs=2, space="PSUM") as cxp,
            tc.tile_pool(name="rope", bufs=3) as ropep,
            tc.tile_pool(name="vaug", bufs=1) as vaugp,
            tc.tile_pool(name="expt", bufs=4) as exptp,
            tc.tile_pool(name="ctx", bufs=1) as ctxp,
            tc.tile_pool(name="nrm", bufs=3) as nrmp,
            tc.tile_pool(name="outs", bufs=3) as outsp,
            tc.tile_pool(name="zdr", bufs=4, space="DRAM") as zdrp,
        ):
            # ---- persistent weights / tables ----
            wq_sb = wts.tile([128, 8, JC], F32R, tag="wq")
            nc.sync.dma_start(
                out=wq_sb[:], in_=wqT[:, :].rearrange("(k p) j -> p k j", p=128)
            )
            wk_sb = wts.tile([128, 8, JC], F32R, tag="wk")
            nc.sync.dma_start(
                out=wk_sb[:], in_=wkT[:, :].rearrange("(k p) j -> p k j", p=128)
            )
            wv_sb = wts.tile([128, 8, JC], F32R, tag="wv")
            nc.sync.dma_start(
                out=wv_sb[:], in_=wvT[:, :].rearrange("(k p) j -> p k j", p=128)
            )
            ident = tabs.tile([128, 128], F32, tag="ident")
            make_identity(nc, ident[:])
            onesc = tabs.tile([128, NKT], F32, tag="ones")
            nc.vector.memset(onesc[:], 1.0)

            for b in range(B):
                # ======== QKV projections (+RoPE), feature-major ========
                qT = qkvp.tile([128, S], F16, tag="qT")
                kT = qkvp.tile([128, S], F16, tag="kT")
                vT = qkvp.tile([128, S], F32, tag="vT")

                chains = []
                for nchi in range(S // TQB):
                    for kind, w_sb in (("q", wq_sb), ("k", wk_sb), ("v", wv_sb)):
                        chains.append((kind, w_sb, nchi))
                chunk_cache = {}

                def get_chunk(k, nchi):
                    if (k, nchi) not in chunk_cache:
                        t0 = b * S + nchi * TQB
                        c = hst.tile([128, TQB], F32R, tag="hst")
                        nc.sync.dma_start(
                            out=c[:], in_=hsT[128 * k : 128 * (k + 1), t0 : t0 + TQB]
                        )
                        chunk_cache[(k, nchi)] = c
                    return chunk_cache[(k, nchi)]

                for i0 in range(0, len(chains), 2):
                    pair = chains[i0 : i0 + 2]
                    pt_a = cxp.tile([128, TQB], F32, tag="cx")
                    pt_b = cxp.tile([128, TQB], F32, tag="cx")
                    ptiles = [pt_a, pt_b][: len(pair)]
                    for k in range(8):
                        for (kind, w_sb, nchi), p in zip(pair, ptiles):
                            nc.tensor.matmul(
                                p[:], w_sb[:, k, :], get_chunk(k, nchi)[:],
                                start=(k == 0), stop=(k == 7),
                            )
                    if b == 0 and i0 == 0:
                        cos_sb = tabs.tile([128, S], F32, tag="cos")
                        nc.sync.dma_start(out=cos_sb[:], in_=cosT[:, :])
                        sin_sb = tabs.tile([128, S], F32, tag="sin")
                        nc.sync.dma_start(out=sin_sb[:], in_=sinTs[:, :])
                    for (kind, w_sb, nchi), p in zip(pair, ptiles):
                        sl = slice(nchi * TQB, (nchi + 1) * TQB)
                        if kind == "v":
                            nc.vector.tensor_copy(vT[:, sl], p[:])
                            continue
                        dstT = qT if kind == "q" else kT
                        raw = ropep.tile([128, TQB], F32, tag="raw")
                        nc.vector.tensor_copy(raw[:], p[:])
                        rot = ropep.tile([128, TQB], F32, tag="rot")
                        for h0 in (0, 64):
                            nc.sync.dma_start(
                                out=rot[h0 : h0 + 32, :], in_=raw[h0 + 32 : h0 + 64, :]
                            )
                            nc.sync.dma_start(
                                out=rot[h0 + 32 : h0 + 64, :], in_=raw[h0 : h0 + 32, :]
                            )
                        t1 = ropep.tile([128, TQB], F32, tag="t1")
                        nc.vector.tensor_mul(t1[:], raw[:], cos_sb[:, sl])
                        t2 = ropep.tile([128, TQB], F32, tag="t2")
                        nc.vector.tensor_mul(t2[:], rot[:], sin_sb[:, sl])
                        nc.vector.tensor_add(dstT[:, sl], t1[:], t2[:])

                if b == 0:
                    wJ = wts.tile([128, H], F32R, tag="wj")
                    nc.sync.dma_start(out=wJ[:], in_=woJI[:, :])

                # zero-padded per-head K so scores run full-array K=128
                kZA = qkvp.tile([128, S], F16, tag="kZA")
                nc.vector.memset(kZA[64:128, :], 0.0)
                nc.vector.tensor_copy(kZA[0:64, :], kT[0:64, :])
                kZB = qkvp.tile([128, S], F16, tag="kZB")
                nc.vector.memset(kZB[0:64, :], 0.0)
                nc.vector.tensor_copy(kZB[64:128, :], kT[64:128, :])

                # ======== v transpose -> per-head augmented V (M padded to 128) ====
                vA = vaugp.tile([128, NKT, 128], F16, tag="vA")
                vB = vaugp.tile([128, NKT, 128], F16, tag="vB")
                nc.vector.memset(vA[:, :, 65:128], 0.0)
                nc.vector.memset(vB[:, :, 65:128], 0.0)
                nc.vector.tensor_copy(vA[:, :, 64], onesc[:])
                nc.vector.tensor_copy(vB[:, :, 64], onesc[:])
                for tkb in range(NKT):
                    pt = ps.tile([128, WAVE], F32, tag="ps")
                    nc.tensor.transpose(
                        pt[:, 0:128], vT[:, 128 * tkb : 128 * (tkb + 1)], ident[:]
                    )
                    nc.vector.tensor_copy(vA[:, tkb, 0:64], pt[:, 0:64])
                    nc.vector.tensor_copy(vB[:, tkb, 0:64], pt[:, 64:128])

                # ======== attention: scoresT -> exp -> ctxT ========
                ctxS = ctxp.tile([128, S], F32R, tag="cts")
                ctxB = ctxp.tile([64, S], F32R, tag="ctb")
                ctxA = ctxS
                for tqb in range(S // TQB):
                    qsl = slice(tqb * TQB, (tqb + 1) * TQB)
                    cxA = cxp.tile([128, TQB], F32, tag="cx")
                    cxB = cxp.tile([128, TQB], F32, tag="cx")
                    for p in range(NKT // 2):
                        scA = ps.tile([128, 2 * TQB], F32, tag="ps")
                        scB = ps.tile([128, 2 * TQB], F32, tag="ps")
                        for t in range(2):
                            tkb = 2 * p + t
                            ksl = slice(128 * tkb, 128 * (tkb + 1))
                            nc.tensor.matmul(
                                scA[:, t * TQB : (t + 1) * TQB],
                                kZA[:, ksl], qT[:, qsl],
                                start=True, stop=True,
                            )
                            nc.tensor.matmul(
                                scB[:, t * TQB : (t + 1) * TQB],
                                kZB[:, ksl], qT[:, qsl],
                                start=True, stop=True,
                            )
                        etA = exptp.tile([128, 2 * TQB], F16, tag="et")
                        nc.scalar.activation(etA[:], scA[:], EXP, scale=0.125)
                        etB = exptp.tile([128, 2 * TQB], F16, tag="et")
                        nc.scalar.activation(etB[:], scB[:], EXP, scale=0.125)
                        for t in range(2):
                            tkb = 2 * p + t
                            st, sp = tkb == 0, tkb == NKT - 1
                            tsl = slice(t * TQB, (t + 1) * TQB)
                            nc.tensor.matmul(
                                cxA[:, :], vA[:, tkb, :], etA[:, tsl],
                                start=st, stop=sp,
                            )
                            nc.tensor.matmul(
                                cxB[:, :], vB[:, tkb, :], etB[:, tsl],
                                start=st, stop=sp,
                            )
                    for cx, ctxT in ((cxA, ctxS), (cxB, ctxB)):
                        craw = nrmp.tile([65, TQB], F32, tag="craw")
                        nc.vector.tensor_copy(craw[:], cx[0:65, :])
                        rzf = nrmp.tile([1, TQB], F32, tag="rzf")
                        nc.vector.reciprocal(rzf[:], craw[64:65, :])
                        zd = zdrp.tile([1, TQB], F32, tag="zd")
                        nc.sync.dma_start(out=zd[:], in_=rzf[:])
                        zrep = nrmp.tile([64, TQB], F32, tag="zrep")
                        nc.sync.dma_start(
                            out=zrep[:], in_=zd[0:1, :].to_broadcast([64, TQB])
                        )
                        dst = ctxT[0:64, qsl] if ctxT is ctxS else ctxT[:, qsl]
                        nc.vector.tensor_mul(dst, craw[0:64, :], zrep[:])
                        if ctxT is ctxB:
                            nc.sync.dma_start(
                                out=ctxS[64:128, qsl], in_=ctxB[:, qsl]
                            )

                # ======== output projection (natural-layout out) ========
                for tq8 in range(S // 128):
                    po = ps.tile([128, WAVE], F32, tag="ps")
                    csl = slice(128 * tq8, 128 * (tq8 + 1))
                    for ich in range(2):
                        isl = slice(ich * 512, (ich + 1) * 512)
                        nc.tensor.matmul(
                            po[:, isl], ctxS[:, csl], wJ[:, isl], start=True, stop=True
                        )
                    ot = outsp.tile([128, H], F32, tag="ot")
                    if tq8 % 2 == 0:
                        nc.vector.tensor_copy(ot[:], po[:])
                    else:
                        nc.scalar.copy(ot[:], po[:])
                    nc.sync.dma_start(
                        out=out[b * S + 128 * tq8 : b * S + 128 * (tq8 + 1), :],
                        in_=ot[:],
                    )

    nc.compile()
    return nc


def _rope_tables():
    inv_freq = 1.0 / (BASE ** (np.arange(0, HD, 2, dtype=np.float64) / HD))
    t = np.arange(S, dtype=np.float64)
    freqs = np.outer(t, inv_freq)  # [S, 32]
    emb = np.concatenate([freqs, freqs], -1)  # [S, 64]
    cos = np.cos(emb).T.astype(np.float32)  # [64, S]
    sin = np.sin(emb).T.astype(np.float32)
    sin_signed = sin.copy()
    sin_signed[0:32] = -sin_signed[0:32]
    cosT = np.ascontiguousarray(np.tile(cos, (2, 1)))  # [128, S]
    sinTs = np.ascontiguousarray(np.tile(sin_signed, (2, 1)))
    return cosT, sinTs


def kernel(hidden_states, Wq, Wk, Wv, Wo):
    hidden_states = np.asarray(hidden_states, np.float32)
    Wq, Wk, Wv, Wo = (np.asarray(w, np.float32) for w in (Wq, Wk, Wv, Wo))

    if _nc_cache[0] is None:
        _nc_cache[0] = _build()
    nc = _nc_cache[0]

    hsT = np.ascontiguousarray(hidden_states.reshape(T, H).T)  # [H, T]
    cosT, sinTs = _rope_tables()
    in_maps = []
    for c in range(NCORES):
        sl = slice(JC * c, JC * (c + 1))
        in_maps.append(
            {
                "hsT": hsT,
                "wqT": np.ascontiguousarray(Wq[sl, :].T),
                "wkT": np.ascontiguousarray(Wk[sl, :].T),
                "wvT": np.ascontiguousarray(Wv[sl, :].T),
                "woJI": np.ascontiguousarray(Wo[:, sl].T),
                "cosT": cosT,
                "sinTs": sinTs,
            }
        )

    from concourse.bass_utils import run_bass_kernel_spmd

    res = run_bass_kernel_spmd(nc, in_maps, core_ids=list(range(NCORES)))
    acc = np.zeros((T, H), np.float64)
    for c in range(NCORES):
        acc += res.results[c]["out"]
    return acc.astype(np.float32).reshape(B, S, H)



# revision 5
# speedup vs baseline: 1.1408x; 1.1408x over previous
"""Multi-head attention (B=2, S=2048, H=1024, NH=16, HD=64) on 8 trn2 cores.

Sharding: tensor-parallel over heads. Core c owns heads {2c, 2c+1}, i.e.
feature columns [128c, 128c+128) of q/k/v. Wq/Wk/Wv are column-sharded,
Wo row-sharded; each core computes a full-shape partial output (fp16) and
the host sums the 8 partials (the row-parallel reduce) during unshard.

v2 restructure vs the first working kernel:
- fp16 weights/activations everywhere (fp32 accumulation in PSUM): halves
  HBM traffic and keeps all matmuls at 1 cycle/row.
- Cross-batch software pipelining: the attention phase is ACT-bound (exp
  of 16.8M scores/core = ~146us on the scalar engine), so QKV projection
  matmuls of the *other* batch and finished-tile output projections are
  interleaved into the attention instruction stream as PE filler work.
  This keeps the PE dense (HAM clock gate stays at 2.4GHz) and hides the
  projection phases under the exp bottleneck.
- Softmax normalizer: augmented-V ones column gives z = sum(exp) in the
  ctx matmul; 1/z computed once per (tqb, head) in fp16 on DVE and
  replicated across 64 partitions with a tiny PE broadcast matmul
  (no DRAM-roundtrip broadcast DMAs).
- RoPE in fp16 on DVE (2x rate); the k-RoPE adds write directly into the
  zero-padded kZA/kZB score stationaries.
- PSUM->SBUF evacuations placed per phase: ACT during QKV(b0) and the
  tail, DVE during the ACT-saturated attention windows.
"""

from collections import deque

import numpy as np

B, S, H, NH, HD = 2, 2048, 1024, 16, 64
NCORES = 8
JC = 128  # head-columns per core (2 heads x 64)
T = B * S  # 4096 tokens
TQB = 512  # tq block
NKT = S // 128  # 16 tk blocks per batch
CH = 512  # qkv projection token-chunk
NCH = S // CH  # 4 chunks per batch
BASE = 10000.0

_nc_cache = [None]


def _build():
    import concourse.tile as tile
    from concourse import bacc, mybir
    from concourse.masks import make_identity

    F32 = mybir.dt.float32
    F16 = mybir.dt.float16
    EXP = mybir.ActivationFunctionType.Exp

    nc = bacc.Bacc("TRN2", target_bir_lowering=False, debug=False)

    hsT = nc.dram_tensor("hsT", [H, T], F16, kind="ExternalInput").ap()
    wqT = nc.dram_tensor("wqT", [H, JC], F16, kind="ExternalInput").ap()
    wkT = nc.dram_tensor("wkT", [H, JC], F16, kind="ExternalInput").ap()
    wvT = nc.dram_tensor("wvT", [H, JC], F16, kind="ExternalInput").ap()
    woJI = nc.dram_tensor("woJI", [JC, H], F16, kind="ExternalInput").ap()
    cosT = nc.dram_tensor("cosT", [128, S], F16, kind="ExternalInput").ap()
    sinTs = nc.dram_tensor("sinTs", [128, S], F16, kind="ExternalInput").ap()
    out = nc.dram_tensor("out", [T, H], F16, kind="ExternalOutput").ap()

    with tile.TileContext(nc) as tc:
        with (
            tc.tile_pool(name="wts", bufs=1) as wts,
            tc.tile_pool(name="tabs", bufs=1) as tabs,
            tc.tile_pool(name="hst", bufs=8) as hstp,
            tc.tile_pool(name="perb", bufs=2) as perb,
            tc.tile_pool(name="rope", bufs=3) as ropep,
            tc.tile_pool(name="expt", bufs=4) as exptp,
            tc.tile_pool(name="nrm", bufs=3) as nrmp,
            tc.tile_pool(name="outs", bufs=3) as outsp,
            tc.tile_pool(name="sc", bufs=2, space="PSUM") as scp,
            tc.tile_pool(name="cx", bufs=2, space="PSUM") as cxp,
            tc.tile_pool(name="fill", bufs=2, space="PSUM") as fillp,
        ):
            # ---- persistent weights / tables ----
            wq_sb = wts.tile([128, 8, JC], F16, tag="wq")
            nc.sync.dma_start(
                out=wq_sb[:], in_=wqT[:, :].rearrange("(k p) j -> p k j", p=128)
            )
            wk_sb = wts.tile([128, 8, JC], F16, tag="wk")
            nc.sync.dma_start(
                out=wk_sb[:], in_=wkT[:, :].rearrange("(k p) j -> p k j", p=128)
            )
            wv_sb = wts.tile([128, 8, JC], F16, tag="wv")
            nc.sync.dma_start(
                out=wv_sb[:], in_=wvT[:, :].rearrange("(k p) j -> p k j", p=128)
            )
            wJ = wts.tile([128, H], F16, tag="wj")
            nc.sync.dma_start(out=wJ[:], in_=woJI[:, :])
            cos_sb = tabs.tile([128, S], F16, tag="cos")
            nc.sync.dma_start(out=cos_sb[:], in_=cosT[:, :])
            sin_sb = tabs.tile([128, S], F16, tag="sin")
            nc.sync.dma_start(out=sin_sb[:], in_=sinTs[:, :])
            ident = tabs.tile([128, 128], F16, tag="ident")
            make_identity(nc, ident[:])
            ones16 = tabs.tile([1, 64], F16, tag="ones")
            nc.vector.memset(ones16[:], 1.0)

            # ---- per-batch state ----
            st = [dict() for _ in range(B)]

            def load_hst(b):
                for nchi in range(NCH):
                    c = hstp.tile([128, 8, CH], F16, tag="hst", name="hst")
                    t0 = b * S + nchi * CH
                    nc.sync.dma_start(
                        out=c[:],
                        in_=hsT[:, t0 : t0 + CH].rearrange(
                            "(k p) t -> p k t", p=128
                        ),
                    )
                    st[b][("hst", nchi)] = c

            def init_batch(b):
                sb = st[b]
                sb["qT"] = perb.tile([128, S], F16, tag="qT", name="qT")
                sb["kZA"] = perb.tile([128, S], F16, tag="kZA", name="kZA")
                sb["kZB"] = perb.tile([128, S], F16, tag="kZB", name="kZB")
                sb["vT"] = perb.tile([128, S], F16, tag="vT", name="vT")
                sb["vA"] = perb.tile([128, NKT, 128], F16, tag="vA", name="vA")
                sb["vB"] = perb.tile([128, NKT, 128], F16, tag="vB", name="vB")
                sb["ctxS"] = perb.tile([128, S], F16, tag="ctxS", name="ctxS")
                sb["ctxB"] = perb.tile([64, S], F16, tag="ctxB", name="ctxB")
                nc.gpsimd.memset(sb["kZA"][64:128, :], 0.0)
                nc.gpsimd.memset(sb["kZB"][0:64, :], 0.0)
                nc.gpsimd.memset(sb["vA"][:, :, 64:65], 1.0)
                nc.gpsimd.memset(sb["vA"][:, :, 65:128], 0.0)
                nc.gpsimd.memset(sb["vB"][:, :, 64:65], 1.0)
                nc.gpsimd.memset(sb["vB"][:, :, 65:128], 0.0)

            def copy_evac(dst, src, b):
                # PSUM->SBUF evacuation: ACT when the scalar engine is
                # idle (b=0 QKV phase), DVE when it's exp-saturated.
                if b == 0:
                    nc.scalar.copy(dst, src)
                else:
                    nc.vector.tensor_copy(dst, src)

            def qkv_chain(b, kind, nchi):
                sb = st[b]
                w_sb = {"q": wq_sb, "k": wk_sb, "v": wv_sb}[kind]
                pt = fillp.tile([128, CH], F32, tag="fill")
                chunk = sb[("hst", nchi)]
                for k in range(8):
                    nc.tensor.matmul(
                        pt[:], w_sb[:, k, :], chunk[:, k, :],
                        start=(k == 0), stop=(k == 7),
                    )
                sl = slice(nchi * CH, (nchi + 1) * CH)
                if kind == "v":
                    copy_evac(sb["vT"][:, sl], pt[:], b)
                    return
                raw = ropep.tile([128, CH], F16, tag="raw")
                copy_evac(raw[:], pt[:], b)
                rot = ropep.tile([128, CH], F16, tag="rot")
                for h0 in (0, 64):
                    nc.sync.dma_start(
                        out=rot[h0 : h0 + 32, :], in_=raw[h0 + 32 : h0 + 64, :]
                    )
                    nc.sync.dma_start(
                        out=rot[h0 + 32 : h0 + 64, :], in_=raw[h0 : h0 + 32, :]
                    )
                t1 = ropep.tile([128, CH], F16, tag="t1")
                nc.vector.tensor_mul(t1[:], raw[:], cos_sb[:, sl])
                t2 = ropep.tile([128, CH], F16, tag="t2")
                nc.vector.tensor_mul(t2[:], rot[:], sin_sb[:, sl])
                if kind == "q":
                    nc.vector.tensor_add(sb["qT"][:, sl], t1[:], t2[:])
                else:
                    nc.vector.tensor_add(
                        sb["kZA"][0:64, sl], t1[0:64, :], t2[0:64, :]
                    )
                    nc.vector.tensor_add(
                        sb["kZB"][64:128, sl], t1[64:128, :], t2[64:128, :]
                    )

            def vtrans(b, nchi):
                sb = st[b]
                for i in range(CH // 128):
                    tkb = nchi * (CH // 128) + i
                    vtr = fillp.tile([128, 128], F16, tag="fill", name="vtr")
                    nc.tensor.transpose(
                        vtr[:], sb["vT"][:, 128 * tkb : 128 * (tkb + 1)], ident[:]
                    )
                    nc.vector.tensor_copy(sb["vA"][:, tkb, 0:64], vtr[:, 0:64])
                    nc.vector.tensor_copy(sb["vB"][:, tkb, 0:64], vtr[:, 64:128])

            def out_proj(b, tq8, ich):
                sb = st[b]
                po = fillp.tile([128, CH], F32, tag="fill")
                isl = slice(ich * 512, (ich + 1) * 512)
                nc.tensor.matmul(
                    po[:],
                    sb["ctxS"][:, 128 * tq8 : 128 * (tq8 + 1)],
                    wJ[:, isl],
                    start=True, stop=True,
                )
                ot = outsp.tile([128, 512], F16, tag="ot")
                if b == 0:
                    nc.vector.tensor_copy(ot[:], po[:])
                else:
                    nc.scalar.copy(ot[:], po[:])
                nc.sync.dma_start(
                    out=out[b * S + 128 * tq8 : b * S + 128 * (tq8 + 1), isl],
                    in_=ot[:],
                )

            # ---- filler machinery: units of deferred PE-centric work ----
            fillers = deque()

            def pop_filler(n=1):
                for _ in range(n):
                    if fillers:
                        fillers.popleft()()

            def flush_fillers():
                while fillers:
                    fillers.popleft()()

            # ---- attention ----
            def attn(b):
                sb = st[b]
                qT, kZA, kZB = sb["qT"], sb["kZA"], sb["kZB"]
                vA, vB = sb["vA"], sb["vB"]
                pending_norm = []

                def norm_tail(cx_pairs, tqb):
                    # deferred: PE broadcast of 1/z + DVE normalize + push
                    # out-proj units for the finished tq block
                    def run():
                        qsl = slice(tqb * TQB, (tqb + 1) * TQB)
                        for craw, rz1, which in cx_pairs:
                            rb = fillp.tile([64, TQB], F32, tag="fill", name="rb")
                            nc.tensor.matmul(
                                rb[:], ones16[:], rz1[:], start=True, stop=True
                            )
                            if which == "A":
                                dst = sb["ctxS"][0:64, qsl]
                            else:
                                dst = sb["ctxB"][:, qsl]
                            nc.vector.tensor_mul(dst, craw[0:64, :], rb[:])
                        nc.sync.dma_start(
                            out=sb["ctxS"][64:128, qsl], in_=sb["ctxB"][:, qsl]
                        )
                        for i in range(TQB // 128):
                            tq8 = tqb * (TQB // 128) + i
                            for ich in range(2):
                                fillers.append(
                                    lambda tq8=tq8, ich=ich: out_proj(b, tq8, ich)
                                )
                    return run

                for tqb in range(4):
                    qsl = slice(tqb * TQB, (tqb + 1) * TQB)
                    cxA = cxp.tile([128, TQB], F32, tag="cx")
                    cxB = cxp.tile([128, TQB], F32, tag="cx")
                    prev = None
                    for p in range(NKT):
                        ksl = slice(128 * p, 128 * (p + 1))
                        sc = scp.tile([128, 2 * TQB], F32, tag="sc")
                        nc.tensor.matmul(
                            sc[:, 0:TQB], kZA[:, ksl], qT[:, qsl],
                            start=True, stop=True,
                        )
                        nc.tensor.matmul(
                            sc[:, TQB:], kZB[:, ksl], qT[:, qsl],
                            start=True, stop=True,
                        )
                        et = exptp.tile([128, 2 * TQB], F16, tag="et")
                        nc.scalar.activation(et[:], sc[:], EXP, scale=0.125)
                        if prev is not None:
                            pet, pp = prev
                            nc.tensor.matmul(
                                cxA[:], vA[:, pp, :], pet[:, 0:TQB],
                                start=(pp == 0), stop=False,
                            )
                            nc.tensor.matmul(
                                cxB[:], vB[:, pp, :], pet[:, TQB:],
                                start=(pp == 0), stop=False,
                            )
                        prev = (et, p)
                        if pending_norm and p == 2:
                            pending_norm.pop(0)()
                        if p % 2 == 1:
                            pop_filler(1)
                    # flush last ctx (stop=True closes the accumulation)
                    pet, pp = prev
                    nc.tensor.matmul(
                        cxA[:], vA[:, pp, :], pet[:, 0:TQB],
                        start=False, stop=True,
                    )
                    nc.tensor.matmul(
                        cxB[:], vB[:, pp, :], pet[:, TQB:],
                        start=False, stop=True,
                    )
                    # evacuate + reciprocal on DVE; broadcast/normalize deferred
                    pairs = []
                    for cx, which in ((cxA, "A"), (cxB, "B")):
                        craw = nrmp.tile([128, TQB], F16, tag="craw")
                        nc.vector.tensor_copy(craw[:], cx[:])
                        rz1 = nrmp.tile([1, TQB], F16, tag="rz")
                        with nc.allow_low_precision(reason="1/z in fp16 is ok"):
                            nc.vector.reciprocal(rz1[:], craw[64:65, :])
                        pairs.append((craw, rz1, which))
                    pending_norm.append(norm_tail(pairs, tqb))
                while pending_norm:
                    pending_norm.pop(0)()

            # ================= schedule =================
            load_hst(0)
            init_batch(0)
            for nchi in range(NCH):
                for kind in ("q", "k", "v"):
                    qkv_chain(0, kind, nchi)
                vtrans(0, nchi)

            load_hst(1)
            init_batch(1)
            for nchi in range(NCH):
                for kind in ("q", "k", "v"):
                    fillers.append(
                        lambda kind=kind, nchi=nchi: qkv_chain(1, kind, nchi)
                    )
                fillers.append(lambda nchi=nchi: vtrans(1, nchi))

            attn(0)
            flush_fillers()
            attn(1)
            flush_fillers()

    nc.compile()
    return nc


def _rope_tables():
    inv_freq = 1.0 / (BASE ** (np.arange(0, HD, 2, dtype=np.float64) / HD))
    t = np.arange(S, dtype=np.float64)
    freqs = np.outer(t, inv_freq)  # [S, 32]
    emb = np.concatenate([freqs, freqs], -1)  # [S, 64]
    cos = np.cos(emb).T  # [64, S]
    sin = np.sin(emb).T
    sin_signed = sin.copy()
    sin_signed[0:32] = -sin_signed[0:32]
    cosT = np.ascontiguousarray(np.tile(cos, (2, 1))).astype(np.float16)
    sinTs = np.ascontiguousarray(np.tile(sin_signed, (2, 1))).astype(np.float16)
    return cosT, sinTs


def _in_maps(hidden_states, Wq, Wk, Wv, Wo):
    hsT = np.ascontiguousarray(
        hidden_states.reshape(T, H).T.astype(np.float16)
    )  # [H, T]
    cosT, sinTs = _rope_tables()
    maps = []
    for c in range(NCORES):
        sl = slice(JC * c, JC * (c + 1))
        maps.append(
            {
                "hsT": hsT,
                "wqT": np.ascontiguousarray(Wq[sl, :].T.astype(np.float16)),
                "wkT": np.ascontiguousarray(Wk[sl, :].T.astype(np.float16)),
                "wvT": np.ascontiguousarray(Wv[sl, :].T.astype(np.float16)),
                "woJI": np.ascontiguousarray(Wo[:, sl].T.astype(np.float16)),
                "cosT": cosT,
                "sinTs": sinTs,
            }
        )
    return maps


def kernel(hidden_states, Wq, Wk, Wv, Wo):
    hidden_states = np.asarray(hidden_states, np.float32)
    Wq, Wk, Wv, Wo = (np.asarray(w, np.float32) for w in (Wq, Wk, Wv, Wo))

    if _nc_cache[0] is None:
        _nc_cache[0] = _build()
    nc = _nc_cache[0]

    from concourse.bass_utils import run_bass_kernel_spmd

    in_maps = _in_maps(hidden_states, Wq, Wk, Wv, Wo)
    res = run_bass_kernel_spmd(nc, in_maps, core_ids=list(range(NCORES)))
    acc = np.zeros((T, H), np.float32)
    for c in range(NCORES):
        acc += res.results[c]["out"].astype(np.float32)
    return acc.reshape(B, S, H)


# revision 8
# speedup vs baseline: 1.3660x; 1.1974x over previous
"""Multi-head attention (B=2, S=2048, H=1024, NH=16, HD=64) on 8 trn2 cores.

Sharding: tensor-parallel over heads. Core c owns heads {2c, 2c+1}, i.e.
feature columns [128c, 128c+128) of q/k/v. Wq/Wk/Wv are column-sharded,
Wo row-sharded; each core computes a full-shape partial output (fp16) and
the host sums the 8 partials (the row-parallel reduce) during unshard.

v2 restructure vs the first working kernel:
- fp16 weights/activations everywhere (fp32 accumulation in PSUM): halves
  HBM traffic and keeps all matmuls at 1 cycle/row.
- Cross-batch software pipelining: the attention phase is ACT-bound (exp
  of 16.8M scores/core = ~146us on the scalar engine), so QKV projection
  matmuls of the *other* batch and finished-tile output projections are
  interleaved into the attention instruction stream as PE filler work.
  This keeps the PE dense (HAM clock gate stays at 2.4GHz) and hides the
  projection phases under the exp bottleneck.
- Softmax normalizer: augmented-V ones column gives z = sum(exp) in the
  ctx matmul; 1/z computed once per (tqb, head) in fp16 on DVE and
  replicated across 64 partitions with a tiny PE broadcast matmul
  (no DRAM-roundtrip broadcast DMAs).
- RoPE in fp16 on DVE (2x rate); the k-RoPE adds write directly into the
  zero-padded kZA/kZB score stationaries.
- PSUM->SBUF evacuations placed per phase: ACT during QKV(b0) and the
  tail, DVE during the ACT-saturated attention windows.
"""

from collections import deque

import numpy as np

B, S, H, NH, HD = 2, 2048, 1024, 16, 64
NCORES = 8
JC = 128  # head-columns per core (2 heads x 64)
T = B * S  # 4096 tokens
TQB = 512  # tq block
NKT = S // 128  # 16 tk blocks per batch
CH = 512  # qkv projection token-chunk
NCH = S // CH  # 4 chunks per batch
BASE = 10000.0

_nc_cache = [None]


def _build():
    import concourse.tile as tile
    from concourse import bacc, mybir
    from concourse.masks import make_identity

    F32 = mybir.dt.float32
    F16 = mybir.dt.float16
    EXP = mybir.ActivationFunctionType.Exp

    nc = bacc.Bacc("TRN2", target_bir_lowering=False, debug=False)

    hsT = nc.dram_tensor("hsT", [H, T], F16, kind="ExternalInput").ap()
    wqT = nc.dram_tensor("wqT", [H, JC], F16, kind="ExternalInput").ap()
    wkT = nc.dram_tensor("wkT", [H, JC], F16, kind="ExternalInput").ap()
    wvT = nc.dram_tensor("wvT", [H, JC], F16, kind="ExternalInput").ap()
    woJI = nc.dram_tensor("woJI", [JC, H], F16, kind="ExternalInput").ap()
    cosT = nc.dram_tensor("cosT", [128, S], F16, kind="ExternalInput").ap()
    sinTs = nc.dram_tensor("sinTs", [128, S], F16, kind="ExternalInput").ap()
    out = nc.dram_tensor("out", [T, H], F16, kind="ExternalOutput").ap()

    with tile.TileContext(nc) as tc:
        with (
            tc.tile_pool(name="wts", bufs=1) as wts,
            tc.tile_pool(name="tabs", bufs=1) as tabs,
            tc.tile_pool(name="hst", bufs=8) as hstp,
            tc.tile_pool(name="perb", bufs=2) as perb,
            tc.tile_pool(name="rope", bufs=3) as ropep,
            tc.tile_pool(name="expt", bufs=4) as exptp,
            tc.tile_pool(name="nrm", bufs=3) as nrmp,
            tc.tile_pool(name="outs", bufs=3) as outsp,
            tc.tile_pool(name="sc", bufs=2, space="PSUM") as scp,
            tc.tile_pool(name="cx", bufs=2, space="PSUM") as cxp,
            tc.tile_pool(name="fill", bufs=2, space="PSUM") as fillp,
        ):
            # ---- per-batch state ----
            st = [dict() for _ in range(B)]

            def load_hst_chunk(b, nchi):
                c = hstp.tile([128, 8, CH], F16, tag="hst", name="hst")
                t0 = b * S + nchi * CH
                nc.sync.dma_start(
                    out=c[:],
                    in_=hsT[:, t0 : t0 + CH].rearrange("(k p) t -> p k t", p=128),
                )
                st[b][("hst", nchi)] = c

            def load_hst(b):
                for nchi in range(NCH):
                    load_hst_chunk(b, nchi)

            # ---- weights / tables (chunk0 + projection weights first so the
            # first QKV chain can start ASAP) ----
            load_hst_chunk(0, 0)
            wq_sb = wts.tile([128, 8, JC], F16, tag="wq")
            nc.sync.dma_start(
                out=wq_sb[:], in_=wqT[:, :].rearrange("(k p) j -> p k j", p=128)
            )
            wk_sb = wts.tile([128, 8, JC], F16, tag="wk")
            nc.sync.dma_start(
                out=wk_sb[:], in_=wkT[:, :].rearrange("(k p) j -> p k j", p=128)
            )
            wv_sb = wts.tile([128, 8, JC], F16, tag="wv")
            nc.sync.dma_start(
                out=wv_sb[:], in_=wvT[:, :].rearrange("(k p) j -> p k j", p=128)
            )
            cos_sb = tabs.tile([128, S], F16, tag="cos")
            nc.sync.dma_start(out=cos_sb[:], in_=cosT[:, :])
            sin_sb = tabs.tile([128, S], F16, tag="sin")
            nc.sync.dma_start(out=sin_sb[:], in_=sinTs[:, :])
            wJ = wts.tile([128, H], F16, tag="wj")
            nc.sync.dma_start(out=wJ[:], in_=woJI[:, :])
            ident = tabs.tile([128, 128], F16, tag="ident")
            make_identity(nc, ident[:])

            def init_batch(b):
                sb = st[b]
                sb["qT"] = perb.tile([128, S], F16, tag="qT", name="qT")
                sb["kZA"] = perb.tile([128, S], F16, tag="kZA", name="kZA")
                sb["kZB"] = perb.tile([128, S], F16, tag="kZB", name="kZB")
                sb["vT"] = perb.tile([128, S], F16, tag="vT", name="vT")
                sb["vA"] = perb.tile([128, NKT, 128], F16, tag="vA", name="vA")
                sb["vB"] = perb.tile([128, NKT, 128], F16, tag="vB", name="vB")
                sb["ctxS"] = perb.tile([128, S], F16, tag="ctxS", name="ctxS")
                sb["ctxB"] = perb.tile([64, S], F16, tag="ctxB", name="ctxB")
                nc.gpsimd.memset(sb["kZA"][64:128, :], 0.0)
                nc.gpsimd.memset(sb["kZB"][0:64, :], 0.0)
                nc.gpsimd.memset(sb["vA"][:, :, 64:65], 1.0)
                nc.gpsimd.memset(sb["vA"][:, :, 65:128], 0.0)
                nc.gpsimd.memset(sb["vB"][:, :, 64:65], 1.0)
                nc.gpsimd.memset(sb["vB"][:, :, 65:128], 0.0)

            def copy_evac(dst, src, b):
                # PSUM->SBUF evacuation: ACT when the scalar engine is
                # idle (b=0 QKV phase), DVE when it's exp-saturated.
                if b == 0:
                    nc.scalar.copy(dst, src)
                else:
                    nc.vector.tensor_copy(dst, src)

            def qkv_chain(b, kind, nchi):
                sb = st[b]
                w_sb = {"q": wq_sb, "k": wk_sb, "v": wv_sb}[kind]
                pt = fillp.tile([128, CH], F32, tag="fill")
                chunk = sb[("hst", nchi)]
                for k in range(8):
                    nc.tensor.matmul(
                        pt[:], w_sb[:, k, :], chunk[:, k, :],
                        start=(k == 0), stop=(k == 7),
                    )
                sl = slice(nchi * CH, (nchi + 1) * CH)
                if kind == "v":
                    copy_evac(sb["vT"][:, sl], pt[:], b)
                    return
                raw = ropep.tile([128, CH], F16, tag="raw")
                copy_evac(raw[:], pt[:], b)
                rot = ropep.tile([128, CH], F16, tag="rot")
                for h0 in (0, 64):
                    nc.sync.dma_start(
                        out=rot[h0 : h0 + 32, :], in_=raw[h0 + 32 : h0 + 64, :]
                    )
                    nc.sync.dma_start(
                        out=rot[h0 + 32 : h0 + 64, :], in_=raw[h0 : h0 + 32, :]
                    )
                t1 = ropep.tile([128, CH], F16, tag="t1")
                nc.vector.tensor_mul(t1[:], raw[:], cos_sb[:, sl])
                t2 = ropep.tile([128, CH], F16, tag="t2")
                nc.vector.tensor_mul(t2[:], rot[:], sin_sb[:, sl])
                if kind == "q":
                    nc.vector.tensor_add(sb["qT"][:, sl], t1[:], t2[:])
                else:
                    nc.vector.tensor_add(
                        sb["kZA"][0:64, sl], t1[0:64, :], t2[0:64, :]
                    )
                    nc.vector.tensor_add(
                        sb["kZB"][64:128, sl], t1[64:128, :], t2[64:128, :]
                    )

            def vtrans(b, nchi):
                sb = st[b]
                for i in range(CH // 128):
                    tkb = nchi * (CH // 128) + i
                    vtr = fillp.tile([128, 128], F16, tag="fill", name="vtr")
                    nc.tensor.transpose(
                        vtr[:], sb["vT"][:, 128 * tkb : 128 * (tkb + 1)], ident[:]
                    )
                    nc.vector.tensor_copy(sb["vA"][:, tkb, 0:64], vtr[:, 0:64])
                    nc.vector.tensor_copy(sb["vB"][:, tkb, 0:64], vtr[:, 64:128])

            def out_proj(b, tq8, ich):
                sb = st[b]
                po = fillp.tile([128, CH], F32, tag="fill")
                isl = slice(ich * 512, (ich + 1) * 512)
                nc.tensor.matmul(
                    po[:],
                    sb["ctxS"][:, 128 * tq8 : 128 * (tq8 + 1)],
                    wJ[:, isl],
                    start=True, stop=True,
                )
                ot = outsp.tile([128, 512], F16, tag="ot")
                if b == 0 or (tq8 + ich) % 2 == 0:
                    nc.vector.tensor_copy(ot[:], po[:])
                else:
                    nc.scalar.copy(ot[:], po[:])
                nc.sync.dma_start(
                    out=out[b * S + 128 * tq8 : b * S + 128 * (tq8 + 1), isl],
                    in_=ot[:],
                )

            # ---- filler machinery: units of deferred PE-centric work ----
            fillers = deque()

            def pop_filler(n=1):
                for _ in range(n):
                    if fillers:
                        fillers.popleft()()

            def flush_fillers():
                while fillers:
                    fillers.popleft()()

            # ---- attention ----
            def attn(b, pop_every=2):
                sb = st[b]
                qT, kZA, kZB = sb["qT"], sb["kZA"], sb["kZB"]
                vA, vB = sb["vA"], sb["vB"]
                def norm_chain(cxA, cxB, tqb):
                    qsl = slice(tqb * TQB, (tqb + 1) * TQB)
                    # z row -> partition-0 tile (reciprocal_approx_fast's
                    # custom-DVE uop requires partition-0-aligned input)
                    zzs = []
                    for cx, which in ((cxA, "A"), (cxB, "B")):
                        zrow = nrmp.tile([1, TQB], F32, tag="zrow")
                        nc.vector.tensor_copy(zrow[:], cx[64:65, :])
                        rz1 = nrmp.tile([1, TQB], F32, tag="rz")
                        nc.vector.reciprocal_approx_fast(out=rz1[:], in_=zrow[:])
                        rzb = nrmp.tile([64, TQB], F32, tag="rzb")
                        nc.gpsimd.partition_broadcast(rzb[:], rz1[:])
                        zzs.append(rzb)
                    for cx, rzb, which in ((cxA, zzs[0], "A"), (cxB, zzs[1], "B")):
                        if which == "A":
                            dst = sb["ctxS"][0:64, qsl]
                        else:
                            dst = sb["ctxB"][:, qsl]
                        nc.vector.tensor_mul(dst, cx[0:64, :], rzb[:])
                    nc.sync.dma_start(
                        out=sb["ctxS"][64:128, qsl], in_=sb["ctxB"][:, qsl]
                    )
                    for i in range(TQB // 128):
                        tq8 = tqb * (TQB // 128) + i
                        for ich in range(2):
                            fillers.append(
                                lambda tq8=tq8, ich=ich: out_proj(b, tq8, ich)
                            )

                for tqb in range(4):
                    if tqb > 0:
                        pop_filler(2)
                    qsl = slice(tqb * TQB, (tqb + 1) * TQB)
                    cxA = cxp.tile([128, TQB], F32, tag="cx")
                    cxB = cxp.tile([128, TQB], F32, tag="cx")
                    prev = None
                    for p in range(NKT):
                        ksl = slice(128 * p, 128 * (p + 1))
                        sc = scp.tile([128, 2 * TQB], F32, tag="sc")
                        nc.tensor.matmul(
                            sc[:, 0:TQB], kZA[:, ksl], qT[:, qsl],
                            start=True, stop=True,
                        )
                        nc.tensor.matmul(
                            sc[:, TQB:], kZB[:, ksl], qT[:, qsl],
                            start=True, stop=True,
                        )
                        et = exptp.tile([128, 2 * TQB], F16, tag="et")
                        nc.scalar.activation(et[:], sc[:], EXP, scale=0.125)
                        if prev is not None:
                            pet, pp = prev
                            nc.tensor.matmul(
                                cxA[:], vA[:, pp, :], pet[:, 0:TQB],
                                start=(pp == 0), stop=False,
                            )
                            nc.tensor.matmul(
                                cxB[:], vB[:, pp, :], pet[:, TQB:],
                                start=(pp == 0), stop=False,
                            )
                        prev = (et, p)
                        if p % pop_every == pop_every - 1:
                            pop_filler(1)
                    # flush last ctx (stop=True closes the accumulation)
                    pet, pp = prev
                    nc.tensor.matmul(
                        cxA[:], vA[:, pp, :], pet[:, 0:TQB],
                        start=False, stop=True,
                    )
                    nc.tensor.matmul(
                        cxB[:], vB[:, pp, :], pet[:, TQB:],
                        start=False, stop=True,
                    )
                    norm_chain(cxA, cxB, tqb)

            # ================= schedule =================
            init_batch(0)
            for nchi in range(NCH):
                if nchi + 1 < NCH:
                    load_hst_chunk(0, nchi + 1)
                for kind in ("q", "k", "v"):
                    qkv_chain(0, kind, nchi)
                vtrans(0, nchi)

            load_hst(1)
            init_batch(1)
            for nchi in range(NCH):
                for kind in ("q", "k", "v"):
                    fillers.append(
                        lambda kind=kind, nchi=nchi: qkv_chain(1, kind, nchi)
                    )
                fillers.append(lambda nchi=nchi: vtrans(1, nchi))

            attn(0)
            flush_fillers()
            attn(1, pop_every=1)
            flush_fillers()

    nc.compile()
    return nc


def _rope_tables():
    inv_freq = 1.0 / (BASE ** (np.arange(0, HD, 2, dtype=np.float64) / HD))
    t = np.arange(S, dtype=np.float64)
    freqs = np.outer(t, inv_freq)  # [S, 32]
    emb = np.concatenate([freqs, freqs], -1)  # [S, 64]
    cos = np.cos(emb).T  # [64, S]
    sin = np.sin(emb).T
    sin_signed = sin.copy()
    sin_signed[0:32] = -sin_signed[0:32]
    cosT = np.ascontiguousarray(np.tile(cos, (2, 1))).astype(np.float16)
    sinTs = np.ascontiguousarray(np.tile(sin_signed, (2, 1))).astype(np.float16)
    return cosT, sinTs


def _in_maps(hidden_states, Wq, Wk, Wv, Wo):
    hsT = np.ascontiguousarray(
        hidden_states.reshape(T, H).T.astype(np.float16)
    )  # [H, T]
    cosT, sinTs = _rope_tables()
    maps = []
    for c in range(NCORES):
        sl = slice(JC * c, JC * (c + 1))
        maps.append(
            {
                "hsT": hsT,
                "wqT": np.ascontiguousarray(Wq[sl, :].T.astype(np.float16)),
                "wkT": np.ascontiguousarray(Wk[sl, :].T.astype(np.float16)),
                "wvT": np.ascontiguousarray(Wv[sl, :].T.astype(np.float16)),
                "woJI": np.ascontiguousarray(Wo[:, sl].T.astype(np.float16)),
                "cosT": cosT,
                "sinTs": sinTs,
            }
        )
    return maps


def kernel(hidden_states, Wq, Wk, Wv, Wo):
    hidden_states = np.asarray(hidden_states, np.float32)
    Wq, Wk, Wv, Wo = (np.asarray(w, np.float32) for w in (Wq, Wk, Wv, Wo))

    if _nc_cache[0] is None:
        _nc_cache[0] = _build()
    nc = _nc_cache[0]

    from concourse.bass_utils import run_bass_kernel_spmd

    in_maps = _in_maps(hidden_states, Wq, Wk, Wv, Wo)
    res = run_bass_kernel_spmd(nc, in_maps, core_ids=list(range(NCORES)))
    acc = np.zeros((T, H), np.float32)
    for c in range(NCORES):
        acc += res.results[c]["out"].astype(np.float32)
    return acc.reshape(B, S, H)


# revision 9
# speedup vs baseline: 1.3875x; 1.0158x over previous
"""Multi-head attention (B=2, S=2048, H=1024, NH=16, HD=64) on 8 trn2 cores.

Sharding: tensor-parallel over heads. Core c owns heads {2c, 2c+1}, i.e.
feature columns [128c, 128c+128) of q/k/v. Wq/Wk/Wv are column-sharded,
Wo row-sharded; each core computes a full-shape partial output (fp16) and
the host sums the 8 partials (the row-parallel reduce) during unshard.

v2 restructure vs the first working kernel:
- fp16 weights/activations everywhere (fp32 accumulation in PSUM): halves
  HBM traffic and keeps all matmuls at 1 cycle/row.
- Cross-batch software pipelining: the attention phase is ACT-bound (exp
  of 16.8M scores/core = ~146us on the scalar engine), so QKV projection
  matmuls of the *other* batch and finished-tile output projections are
  interleaved into the attention instruction stream as PE filler work.
  This keeps the PE dense (HAM clock gate stays at 2.4GHz) and hides the
  projection phases under the exp bottleneck.
- Softmax normalizer: augmented-V ones column gives z = sum(exp) in the
  ctx matmul; 1/z computed once per (tqb, head) in fp16 on DVE and
  replicated across 64 partitions with a tiny PE broadcast matmul
  (no DRAM-roundtrip broadcast DMAs).
- RoPE in fp16 on DVE (2x rate); the k-RoPE adds write directly into the
  zero-padded kZA/kZB score stationaries.
- PSUM->SBUF evacuations placed per phase: ACT during QKV(b0) and the
  tail, DVE during the ACT-saturated attention windows.
"""

from collections import deque

import numpy as np

B, S, H, NH, HD = 2, 2048, 1024, 16, 64
NCORES = 8
JC = 128  # head-columns per core (2 heads x 64)
T = B * S  # 4096 tokens
TQB = 512  # tq block
NKT = S // 128  # 16 tk blocks per batch
CH = 512  # qkv projection token-chunk
NCH = S // CH  # 4 chunks per batch
BASE = 10000.0

_nc_cache = [None]


def _build():
    import concourse.tile as tile
    from concourse import bacc, mybir
    from concourse.masks import make_identity

    F32 = mybir.dt.float32
    F16 = mybir.dt.float16
    EXP = mybir.ActivationFunctionType.Exp

    nc = bacc.Bacc("TRN2", target_bir_lowering=False, debug=False)

    hsT = nc.dram_tensor("hsT", [H, T], F16, kind="ExternalInput").ap()
    wqT = nc.dram_tensor("wqT", [H, JC], F16, kind="ExternalInput").ap()
    wkT = nc.dram_tensor("wkT", [H, JC], F16, kind="ExternalInput").ap()
    wvT = nc.dram_tensor("wvT", [H, JC], F16, kind="ExternalInput").ap()
    woJI = nc.dram_tensor("woJI", [JC, H], F16, kind="ExternalInput").ap()
    cosT = nc.dram_tensor("cosT", [128, S], F16, kind="ExternalInput").ap()
    sinTs = nc.dram_tensor("sinTs", [128, S], F16, kind="ExternalInput").ap()
    out = nc.dram_tensor("out", [T, H], F16, kind="ExternalOutput").ap()

    with tile.TileContext(nc) as tc:
        with (
            tc.tile_pool(name="wts", bufs=1) as wts,
            tc.tile_pool(name="tabs", bufs=1) as tabs,
            tc.tile_pool(name="hst", bufs=8) as hstp,
            tc.tile_pool(name="perb", bufs=2) as perb,
            tc.tile_pool(name="rope", bufs=3) as ropep,
            tc.tile_pool(name="expt", bufs=4) as exptp,
            tc.tile_pool(name="nrm", bufs=3) as nrmp,
            tc.tile_pool(name="outs", bufs=3) as outsp,
            tc.tile_pool(name="sc", bufs=2, space="PSUM") as scp,
            tc.tile_pool(name="cx", bufs=2, space="PSUM") as cxp,
            tc.tile_pool(name="fill", bufs=2, space="PSUM") as fillp,
        ):
            # ---- per-batch state ----
            st = [dict() for _ in range(B)]

            def load_hst_chunk(b, nchi, split=False):
                c = hstp.tile([128, 8, CH], F16, tag="hst", name="hst")
                t0 = b * S + nchi * CH
                src_r = hsT[:, t0 : t0 + CH].rearrange("(k p) t -> p k t", p=128)
                if split:
                    # first k-pair lands fast so the QKV chain starts early
                    nc.sync.dma_start(out=c[:, 0:2, :], in_=src_r[:, 0:2, :])
                    nc.sync.dma_start(out=c[:, 2:8, :], in_=src_r[:, 2:8, :])
                else:
                    nc.sync.dma_start(out=c[:], in_=src_r)
                st[b][("hst", nchi)] = c

            def load_hst(b):
                for nchi in range(NCH):
                    load_hst_chunk(b, nchi)

            # ---- weights / tables (chunk0 + projection weights first so the
            # first QKV chain can start ASAP) ----
            load_hst_chunk(0, 0, split=True)
            wq_sb = wts.tile([128, 8, JC], F16, tag="wq")
            nc.sync.dma_start(
                out=wq_sb[:], in_=wqT[:, :].rearrange("(k p) j -> p k j", p=128)
            )
            wk_sb = wts.tile([128, 8, JC], F16, tag="wk")
            nc.sync.dma_start(
                out=wk_sb[:], in_=wkT[:, :].rearrange("(k p) j -> p k j", p=128)
            )
            wv_sb = wts.tile([128, 8, JC], F16, tag="wv")
            nc.sync.dma_start(
                out=wv_sb[:], in_=wvT[:, :].rearrange("(k p) j -> p k j", p=128)
            )
            cos_sb = tabs.tile([128, S], F16, tag="cos")
            nc.sync.dma_start(out=cos_sb[:], in_=cosT[:, :])
            sin_sb = tabs.tile([128, S], F16, tag="sin")
            nc.sync.dma_start(out=sin_sb[:], in_=sinTs[:, :])
            wJ = wts.tile([128, H], F16, tag="wj")
            nc.sync.dma_start(out=wJ[:], in_=woJI[:, :])
            ident = tabs.tile([128, 128], F16, tag="ident")
            make_identity(nc, ident[:])

            def init_batch(b):
                sb = st[b]
                sb["qT"] = perb.tile([128, S], F16, tag="qT", name="qT")
                sb["kZA"] = perb.tile([128, S], F16, tag="kZA", name="kZA")
                sb["kZB"] = perb.tile([128, S], F16, tag="kZB", name="kZB")
                sb["vT"] = perb.tile([128, S], F16, tag="vT", name="vT")
                sb["vA"] = perb.tile([128, NKT, 128], F16, tag="vA", name="vA")
                sb["vB"] = perb.tile([128, NKT, 128], F16, tag="vB", name="vB")
                sb["ctxS"] = perb.tile([128, S], F16, tag="ctxS", name="ctxS")
                sb["ctxB"] = perb.tile([64, S], F16, tag="ctxB", name="ctxB")
                nc.gpsimd.memset(sb["kZA"][64:128, :], 0.0)
                nc.gpsimd.memset(sb["kZB"][0:64, :], 0.0)
                nc.gpsimd.memset(sb["vA"][:, :, 64:65], 1.0)
                nc.gpsimd.memset(sb["vA"][:, :, 65:128], 0.0)
                nc.gpsimd.memset(sb["vB"][:, :, 64:65], 1.0)
                nc.gpsimd.memset(sb["vB"][:, :, 65:128], 0.0)

            def copy_evac(dst, src, b):
                # PSUM->SBUF evacuation: ACT when the scalar engine is
                # idle (b=0 QKV phase), DVE when it's exp-saturated.
                if b == 0:
                    nc.scalar.copy(dst, src)
                else:
                    nc.vector.tensor_copy(dst, src)

            def qkv_chain(b, kind, nchi):
                sb = st[b]
                w_sb = {"q": wq_sb, "k": wk_sb, "v": wv_sb}[kind]
                pt = fillp.tile([128, CH], F32, tag="fill")
                chunk = sb[("hst", nchi)]
                for k in range(8):
                    nc.tensor.matmul(
                        pt[:], w_sb[:, k, :], chunk[:, k, :],
                        start=(k == 0), stop=(k == 7),
                    )
                sl = slice(nchi * CH, (nchi + 1) * CH)
                if kind == "v":
                    copy_evac(sb["vT"][:, sl], pt[:], b)
                    return
                raw = ropep.tile([128, CH], F16, tag="raw")
                copy_evac(raw[:], pt[:], b)
                rot = ropep.tile([128, CH], F16, tag="rot")
                for h0 in (0, 64):
                    nc.sync.dma_start(
                        out=rot[h0 : h0 + 32, :], in_=raw[h0 + 32 : h0 + 64, :]
                    )
                    nc.sync.dma_start(
                        out=rot[h0 + 32 : h0 + 64, :], in_=raw[h0 : h0 + 32, :]
                    )
                t1 = ropep.tile([128, CH], F16, tag="t1")
                nc.vector.tensor_mul(t1[:], raw[:], cos_sb[:, sl])
                t2 = ropep.tile([128, CH], F16, tag="t2")
                nc.vector.tensor_mul(t2[:], rot[:], sin_sb[:, sl])
                if kind == "q":
                    nc.vector.tensor_add(sb["qT"][:, sl], t1[:], t2[:])
                else:
                    nc.vector.tensor_add(
                        sb["kZA"][0:64, sl], t1[0:64, :], t2[0:64, :]
                    )
                    nc.vector.tensor_add(
                        sb["kZB"][64:128, sl], t1[64:128, :], t2[64:128, :]
                    )

            def vtrans(b, nchi):
                sb = st[b]
                for i in range(CH // 128):
                    tkb = nchi * (CH // 128) + i
                    vtr = fillp.tile([128, 128], F16, tag="fill", name="vtr")
                    nc.tensor.transpose(
                        vtr[:], sb["vT"][:, 128 * tkb : 128 * (tkb + 1)], ident[:]
                    )
                    nc.vector.tensor_copy(sb["vA"][:, tkb, 0:64], vtr[:, 0:64])
                    nc.vector.tensor_copy(sb["vB"][:, tkb, 0:64], vtr[:, 64:128])

            def out_proj(b, tq8, ich):
                sb = st[b]
                po = fillp.tile([128, CH], F32, tag="fill")
                isl = slice(ich * 512, (ich + 1) * 512)
                nc.tensor.matmul(
                    po[:],
                    sb["ctxS"][:, 128 * tq8 : 128 * (tq8 + 1)],
                    wJ[:, isl],
                    start=True, stop=True,
                )
                ot = outsp.tile([128, 512], F16, tag="ot")
                if b == 0 or (tq8 + ich) % 2 == 0:
                    nc.vector.tensor_copy(ot[:], po[:])
                else:
                    nc.scalar.copy(ot[:], po[:])
                nc.sync.dma_start(
                    out=out[b * S + 128 * tq8 : b * S + 128 * (tq8 + 1), isl],
                    in_=ot[:],
                )

            # ---- filler machinery: units of deferred PE-centric work ----
            fillers = deque()  # QKV-type units (consumed during attn b0)
            out_units = deque()  # out-proj units (consumed during attn b1)
            reserve = [0]  # keep N out_units for the tail flush

            def pop_filler(n=1):
                for _ in range(n):
                    if fillers:
                        fillers.popleft()()
                    elif len(out_units) > reserve[0]:
                        out_units.popleft()()

            def flush_fillers():
                while fillers:
                    fillers.popleft()()
                while out_units:
                    out_units.popleft()()

            # ---- attention ----
            def attn(b, pop_every=2):
                sb = st[b]
                qT, kZA, kZB = sb["qT"], sb["kZA"], sb["kZB"]
                vA, vB = sb["vA"], sb["vB"]
                def norm_chain(cxA, cxB, tqb):
                    qsl = slice(tqb * TQB, (tqb + 1) * TQB)
                    # z row -> partition-0 tile (reciprocal_approx_fast's
                    # custom-DVE uop requires partition-0-aligned input)
                    zzs = []
                    for cx, which in ((cxA, "A"), (cxB, "B")):
                        zrow = nrmp.tile([1, TQB], F32, tag="zrow")
                        nc.vector.tensor_copy(zrow[:], cx[64:65, :])
                        rz1 = nrmp.tile([1, TQB], F32, tag="rz")
                        nc.vector.reciprocal_approx_fast(out=rz1[:], in_=zrow[:])
                        rzb = nrmp.tile([64, TQB], F32, tag="rzb")
                        nc.gpsimd.partition_broadcast(rzb[:], rz1[:])
                        zzs.append(rzb)
                    for cx, rzb, which in ((cxA, zzs[0], "A"), (cxB, zzs[1], "B")):
                        if which == "A":
                            dst = sb["ctxS"][0:64, qsl]
                        else:
                            dst = sb["ctxB"][:, qsl]
                        nc.vector.tensor_mul(dst, cx[0:64, :], rzb[:])
                    nc.sync.dma_start(
                        out=sb["ctxS"][64:128, qsl], in_=sb["ctxB"][:, qsl]
                    )
                    for i in range(TQB // 128):
                        tq8 = tqb * (TQB // 128) + i
                        for ich in range(2):
                            out_units.append(
                                lambda tq8=tq8, ich=ich: out_proj(b, tq8, ich)
                            )

                for tqb in range(4):
                    if tqb > 0:
                        pop_filler(2)
                    qsl = slice(tqb * TQB, (tqb + 1) * TQB)
                    cxA = cxp.tile([128, TQB], F32, tag="cx")
                    cxB = cxp.tile([128, TQB], F32, tag="cx")
                    prev = None
                    for p in range(NKT):
                        ksl = slice(128 * p, 128 * (p + 1))
                        sc = scp.tile([128, 2 * TQB], F32, tag="sc")
                        nc.tensor.matmul(
                            sc[:, 0:TQB], kZA[:, ksl], qT[:, qsl],
                            start=True, stop=True,
                        )
                        nc.tensor.matmul(
                            sc[:, TQB:], kZB[:, ksl], qT[:, qsl],
                            start=True, stop=True,
                        )
                        et = exptp.tile([128, 2 * TQB], F16, tag="et")
                        nc.scalar.activation(et[:], sc[:], EXP, scale=0.125)
                        if prev is not None:
                            pet, pp = prev
                            nc.tensor.matmul(
                                cxA[:], vA[:, pp, :], pet[:, 0:TQB],
                                start=(pp == 0), stop=False,
                            )
                            nc.tensor.matmul(
                                cxB[:], vB[:, pp, :], pet[:, TQB:],
                                start=(pp == 0), stop=False,
                            )
                        prev = (et, p)
                        if p % pop_every == pop_every - 1:
                            pop_filler(1)
                    # flush last ctx (stop=True closes the accumulation)
                    pet, pp = prev
                    nc.tensor.matmul(
                        cxA[:], vA[:, pp, :], pet[:, 0:TQB],
                        start=False, stop=True,
                    )
                    nc.tensor.matmul(
                        cxB[:], vB[:, pp, :], pet[:, TQB:],
                        start=False, stop=True,
                    )
                    norm_chain(cxA, cxB, tqb)

            # ================= schedule =================
            init_batch(0)
            for nchi in range(NCH):
                if nchi + 1 < NCH:
                    load_hst_chunk(0, nchi + 1)
                for kind in ("q", "k", "v"):
                    qkv_chain(0, kind, nchi)
                vtrans(0, nchi)

            load_hst(1)
            init_batch(1)
            for nchi in range(NCH):
                for kind in ("q", "k", "v"):
                    fillers.append(
                        lambda kind=kind, nchi=nchi: qkv_chain(1, kind, nchi)
                    )
                fillers.append(lambda nchi=nchi: vtrans(1, nchi))

            attn(0)
            while fillers:
                fillers.popleft()()
            reserve[0] = 8
            attn(1, pop_every=1)
            reserve[0] = 0
            flush_fillers()

    nc.compile()
    return nc


def _rope_tables():
    inv_freq = 1.0 / (BASE ** (np.arange(0, HD, 2, dtype=np.float64) / HD))
    t = np.arange(S, dtype=np.float64)
    freqs = np.outer(t, inv_freq)  # [S, 32]
    emb = np.concatenate([freqs, freqs], -1)  # [S, 64]
    cos = np.cos(emb).T  # [64, S]
    sin = np.sin(emb).T
    sin_signed = sin.copy()
    sin_signed[0:32] = -sin_signed[0:32]
    cosT = np.ascontiguousarray(np.tile(cos, (2, 1))).astype(np.float16)
    sinTs = np.ascontiguousarray(np.tile(sin_signed, (2, 1))).astype(np.float16)
    return cosT, sinTs


def _in_maps(hidden_states, Wq, Wk, Wv, Wo):
    hsT = np.ascontiguousarray(
        hidden_states.reshape(T, H).T.astype(np.float16)
    )  # [H, T]
    cosT, sinTs = _rope_tables()
    maps = []
    for c in range(NCORES):
        sl = slice(JC * c, JC * (c + 1))
        maps.append(
            {
                "hsT": hsT,
                "wqT": np.ascontiguousarray(Wq[sl, :].T.astype(np.float16)),
                "wkT": np.ascontiguousarray(Wk[sl, :].T.astype(np.float16)),
                "wvT": np.ascontiguousarray(Wv[sl, :].T.astype(np.float16)),
                "woJI": np.ascontiguousarray(Wo[:, sl].T.astype(np.float16)),
                "cosT": cosT,
                "sinTs": sinTs,
            }
        )
    return maps


def kernel(hidden_states, Wq, Wk, Wv, Wo):
    hidden_states = np.asarray(hidden_states, np.float32)
    Wq, Wk, Wv, Wo = (np.asarray(w, np.float32) for w in (Wq, Wk, Wv, Wo))

    if _nc_cache[0] is None:
        _nc_cache[0] = _build()
    nc = _nc_cache[0]

    from concourse.bass_utils import run_bass_kernel_spmd

    in_maps = _in_maps(hidden_states, Wq, Wk, Wv, Wo)
    res = run_bass_kernel_spmd(nc, in_maps, core_ids=list(range(NCORES)))
    acc = np.zeros((T, H), np.float32)
    for c in range(NCORES):
        acc += res.results[c]["out"].astype(np.float32)
    return acc.reshape(B, S, H)
